# revision 41
# baseline (speedup 1.0000x reference)
"""CAM (channel attention) module kernel for Trainium2, data-parallel over batch.

Computes, per sample:
    v = x.reshape(C, N)                  # N = H*W
    energy = v @ v.T                     # [C, C]
    att = softmax(rowmax(energy) - energy, axis=-1)
    out = gamma * (att @ v) + x

Distribution: batch B=32 split over 8 NeuronCores (4 samples/core), gamma
replicated.  Per core everything is computed on-chip.  Key structure:

  - E = v v^T is SYMMETRIC: only a (block-)upper-triangular set of strips is
    computed (strip widths 512/384/256/256 >= 256 to stay at full fp32r
    matmul rate); the 5 missing lower 128x128 blocks are filled by PE
    transposes of their mirrors.  Saves ~31% of the energy matmul.
  - softmax(rowmax-e) == exp(rowmin - e)/rowsum, and the row-shift can be any
    GLOBAL constant s as long as exp stays in fp32 range: s = mean(rowmins)-30
    is computed on-chip (rowmins via DVE free-min, summed across partitions
    with a ones-vector matmul).  exp(s - E) applied to E strips IS the
    TRANSPOSED unnormalized attention PT[d,c] (by symmetry of E), so the
    second matmul needs NO attention transpose.
  - Z (row sums) comes for free from an extra ones-column appended to v
    (v tiles are [P, CB*(N+1)] with col N of each block memset to 1), so the
    first output chunk's matmul also yields Z in PSUM col 256.
  - v^T (needed since PE contracts over partitions) is built with PE
    transpose-mode matmuls for the NEXT sample during this sample's softmax
    window and second matmul, keeping PE gap-free.
  - epilogue fuses (psum * (gamma/Z)) + x in one DVE pass.
"""

import sys

sys.path.insert(0, "/opt/trn_rl_repo")

from contextlib import ExitStack

import numpy as np

import concourse.bacc as bacc
import concourse.bass as bass
import concourse.mybir as mybir
import concourse.tile as tile
from concourse import masks
from concourse.bass_utils import run_bass_kernel_spmd

B, C, H, W = 32, 512, 48, 48
N = H * W  # 2304
NP = N + 2  # per-channel-block width in the v tile (+2 ones cols for Z; the
# moving operand of an fp32r matmul must have an even column count)
NCORES = 8
SPC = B // NCORES  # samples per core
P = 128
CB = C // P  # 4 channel blocks
KB = N // P  # 18 spatial chunks of 128
SHIFT_OFF = 30.0  # shift below mean(rowmins): keeps exp(s - E) in fp32 range
STRIP_LO = [0, 128, 256, 256]  # computed d-range start per energy strip
# (src_strip, dst_strip): fill block (dst, src) from transpose of (src, dst)
TFILLS = [(0, 1), (0, 2), (1, 2), (0, 3), (1, 3)]

FP32 = mybir.dt.float32
FP32R = mybir.dt.float32r
AX = mybir.AxisListType.X
OP = mybir.AluOpType
AF = mybir.ActivationFunctionType


def _emit(tc, ctx, x, gamma, out, reps=1):
    nc = tc.nc

    const_pool = ctx.enter_context(tc.tile_pool(name="const", bufs=1))
    ident_f32 = const_pool.tile([P, P], FP32)
    masks.make_identity(nc, ident_f32[:])
    ident = const_pool.tile([P, P], FP32R)
    nc.scalar.copy(ident[:], ident_f32[:])
    gamma_sb = const_pool.tile([P, 1], FP32)
    nc.sync.dma_start(gamma_sb[:], bass.AP(gamma.tensor, 0, [[0, P], [1, 1]]))
    onesm_f32 = const_pool.tile([P, P], FP32)
    nc.gpsimd.memset(onesm_f32[:], 1.0)
    onesm = const_pool.tile([P, P], FP32R)
    nc.scalar.copy(onesm[:], onesm_f32[:])
    ones_f32 = onesm_f32  # [P, >=2] fp32 ones source for the v Z-columns

    v_pool = ctx.enter_context(tc.tile_pool(name="v", bufs=2))
    vt_pool = ctx.enter_context(tc.tile_pool(name="vt", bufs=2))
    pt_pool = ctx.enter_context(tc.tile_pool(name="pt", bufs=1))
    eb_pool = ctx.enter_context(tc.tile_pool(name="ebk", bufs=5))
    o_pool = ctx.enter_context(tc.tile_pool(name="o", bufs=6))
    vec_pool = ctx.enter_context(tc.tile_pool(name="vec", bufs=2))
    # PSUM: 4 banks energy, 2 rotating transpose/misc, 2 rotating output
    ps_e = ctx.enter_context(tc.tile_pool(name="ps_e", bufs=1, space="PSUM"))
    ps_t = ctx.enter_context(tc.tile_pool(name="ps_t", bufs=2, space="PSUM"))
    ps_o = ctx.enter_context(tc.tile_pool(name="ps_o", bufs=2, space="PSUM"))

    nsamp = reps * SPC
    v_t = {}
    vt_t = {}

    def load_v(i, ranges=((0, 768), (768, 1536), (1536, N))):
        # column-range-major order so the first spatial chunks of ALL channel
        # blocks land early (transposes for chunk k need every cb)
        s = i % SPC
        v = v_pool.tile([P, CB * NP], FP32R, tag="v", name=f"v{i}")
        for a, b in ranges:
            for cb in range(CB):
                eng = nc.sync if cb % 2 == 0 else nc.gpsimd
                eng.dma_start(
                    v[:, cb * NP + a : cb * NP + b],
                    x[s, cb * P : (cb + 1) * P, a:b].bitcast(FP32R),
                )
        for cb in range(CB):
            nc.scalar.copy(v[:, cb * NP + N : cb * NP + N + 2], ones_f32[:, 0:2])
        v_t[i] = v

    def a_chunk(j, k, copy_eng):
        # transpose one 128-wide spatial chunk of v(j) into vt(j)
        if k == 0:
            vt_t[j] = vt_pool.tile([P, KB * C], FP32R, tag="vt", name=f"vt{j}")
        v, vt = v_t[j], vt_t[j]
        tps = ps_t.tile([P, 512], FP32R, tag="tps")
        for cb in range(CB):
            nc.tensor.matmul(
                tps[:, cb * P : (cb + 1) * P],
                v[:, cb * NP + k * P : cb * NP + (k + 1) * P],
                ident[:],
                is_transpose=True,
                start=(cb == 0),
                stop=(cb == CB - 1),
            )
        if copy_eng == "dve":
            nc.vector.tensor_copy(vt[:, k * C : (k + 1) * C], tps[:])
        else:
            nc.scalar.copy(vt[:, k * C : (k + 1) * C], tps[:])

    def emit(i):
        s = i % SPC
        v, vt = v_t[i], vt_t.get(i)
        if i + 1 < nsamp:
            load_v(i + 1)
        # one PSUM tile per bank so start=True bank-clears don't create
        # false cross-strip dependencies
        E = [ps_e.tile([P, 512], FP32, tag=f"eb{ib}", name=f"E{ib}") for ib in range(CB)]
        mn_all = vec_pool.tile([P, 2], FP32R, tag="mn")

        def strip(ib):
            lo = STRIP_LO[ib]
            for k in range(KB):
                nc.tensor.matmul(
                    E[ib][:, lo:512],
                    vt[:, k * C + ib * P : k * C + (ib + 1) * P],
                    vt[:, k * C + lo : k * C + C],
                    start=(k == 0),
                    stop=(k == KB - 1),
                )

        def tcopy(xs, ys):
            ebk = eb_pool.tile([P, P], FP32R, tag="ebk", name=f"eb{xs}{ys}")
            nc.scalar.copy(ebk[:], E[xs][:, ys * P : (ys + 1) * P])
            return ebk

        def tfill(xs, ys, ebk):
            nc.tensor.matmul(
                E[ys][:, xs * P : (xs + 1) * P].bitcast(FP32R),
                ebk[:],
                ident[:],
                is_transpose=True,
                start=True,
                stop=True,
            )

        def rmin(ib):
            nc.vector.tensor_reduce(
                mn_all[:, ib : ib + 1], E[ib][:, 0:512], axis=AX, op=OP.min
            )

        # ---- energy: triangular strips, symmetric fills ----
        # the global shift is estimated from strips 0/1's rowmins only (full
        # rows, available mid-B; 256-row mean is within ~1 of the 512-row
        # mean, margins validated), so the softmax chain finishes DURING B
        a_queue = list(range(KB)) if i + 1 < nsamp else []
        cp = {}
        if i == 0:
            # prologue: build vt(0) and accumulate the strips k-major, two
            # chunks behind the transposes, so energy starts while x loads
            for k in range(KB + 2):
                if k < KB:
                    a_chunk(0, k, "dve" if k % 2 else "act")
                    vt = vt_t[0]
                if k >= 2:
                    kk = k - 2
                    for ib in range(CB):
                        lo = STRIP_LO[ib]
                        nc.tensor.matmul(
                            E[ib][:, lo:512],
                            vt[:, kk * C + ib * P : kk * C + (ib + 1) * P],
                            vt[:, kk * C + lo : kk * C + C],
                            start=(kk == 0),
                            stop=(kk == KB - 1),
                        )
            for xs, ys in TFILLS:
                cp[(xs, ys)] = tcopy(xs, ys)
            rmin(0)
            tfill(0, 1, cp[(0, 1)])
            rmin(1)
            for _ in range(min(2, len(a_queue))):
                a_chunk(i + 1, a_queue.pop(0), "act")
        else:
            strip(0)
            for xs, ys in TFILLS:
                if xs == 0:
                    cp[(xs, ys)] = tcopy(xs, ys)
            rmin(0)
            strip(1)
            for xs, ys in TFILLS:
                if xs == 1:
                    cp[(xs, ys)] = tcopy(xs, ys)
            tfill(0, 1, cp[(0, 1)])
            rmin(1)
            strip(2)
        # ---- global shift s = mean(rowmins of strips 0,1) - SHIFT_OFF ----
        # ones[P,P].T @ mn_all replicates the two per-block rowmin sums onto
        # every partition, so no cross-partition broadcast is needed
        srow = ps_t.tile([P, 2], FP32, tag="tps", name="srow")
        nc.tensor.matmul(srow[:], onesm[:], mn_all[:, 0:2], start=True, stop=True)
        ssum = vec_pool.tile([P, 1], FP32, tag="ss")
        nc.vector.tensor_reduce(ssum[:], srow[:, 0:2], axis=AX, op=OP.add)
        s_sb = vec_pool.tile([P, 1], FP32, tag="sb")
        nc.vector.tensor_scalar(
            s_sb[:], ssum[:], 1.0 / 256, -SHIFT_OFF, op0=OP.mult, op1=OP.add
        )
        tfill(0, 2, cp[(0, 2)])
        tfill(1, 2, cp[(1, 2)])
        if i > 0:
            strip(3)
        tfill(0, 3, cp[(0, 3)])
        tfill(1, 3, cp[(1, 3)])

        # ---- PT[d, c] = exp(s - E)  (== transposed unnormalized attention) ----
        pt = pt_pool.tile([P, CB * C], FP32R, tag="pt")

        def exps(cb):
            for db in range(CB):
                nc.scalar.activation(
                    pt[:, db * C + cb * P : db * C + (cb + 1) * P],
                    E[db][:, cb * P : (cb + 1) * P],
                    AF.Exp,
                    bias=s_sb[:],
                    scale=-1.0,
                )

        for cb in range(CB):
            exps(cb)
        for _ in range(min(2, len(a_queue))):
            a_chunk(i + 1, a_queue.pop(0), "dve")

        # ---- out = (PT^T @ [v|1]) * (gamma/Z) + x ----
        spr = 0
        for cb in range(CB):
            # first chunk (spatial 2048..2304) carries the ones column: its
            # PSUM col 256 accumulates Z = rowsum of unnormalized attention
            po = ps_o.tile([P, 512], FP32, tag="po")
            for db in range(CB):
                nc.tensor.matmul(
                    po[:, 0:258],
                    pt[:, db * C + cb * P : db * C + (cb + 1) * P],
                    v[:, db * NP + 2048 : db * NP + N + 2],
                    start=(db == 0),
                    stop=(db == CB - 1),
                )
            r = vec_pool.tile([P, 1], FP32, tag="r")
            nc.vector.reciprocal(r[:], po[:, 256:257])
            scb = vec_pool.tile([P, 1], FP32, tag="scb")
            nc.vector.tensor_tensor(scb[:], r[:], gamma_sb[:], op=OP.mult)
            ot = o_pool.tile([P, 512], FP32, tag="ot", name="ot0")
            nc.vector.scalar_tensor_tensor(
                ot[:, 0:256],
                po[:, 0:256],
                scb[:],
                v[:, cb * NP + 2048 : cb * NP + N].bitcast(FP32),
                op0=OP.mult,
                op1=OP.add,
            )
            nc.sync.dma_start(out[s, cb * P : (cb + 1) * P, 2048:N], ot[:, 0:256])
            for n_off in (0, 512, 1024, 1536):
                if a_queue:
                    spr += 1
                    a_chunk(i + 1, a_queue.pop(0), "dve" if spr % 4 == 3 else "act")
                    po2 = ps_o.tile([P, 512], FP32, tag="po")
                else:
                    # tail (no filler transposes): rotate over 4 PSUM banks so
                    # the po ring never waits on the epilogue drain
                    spr += 1
                    if spr % 2:
                        po2 = ps_o.tile([P, 512], FP32, tag="po")
                    else:
                        po2 = ps_t.tile([P, 512], FP32, tag="tps", name="po_t")
                for db in range(CB):
                    nc.tensor.matmul(
                        po2[:],
                        pt[:, db * C + cb * P : db * C + (cb + 1) * P],
                        v[:, db * NP + n_off : db * NP + n_off + 512],
                        start=(db == 0),
                        stop=(db == CB - 1),
                    )
                ot2 = o_pool.tile([P, 512], FP32, tag="ot", name="ot1")
                nc.vector.scalar_tensor_tensor(
                    ot2[:],
                    po2[:],
                    scb[:],
                    v[:, cb * NP + n_off : cb * NP + n_off + 512].bitcast(FP32),
                    op0=OP.mult,
                    op1=OP.add,
                )
                (nc.gpsimd if n_off % 1024 else nc.sync).dma_start(
                    out[s, cb * P : (cb + 1) * P, n_off : n_off + 512], ot2[:]
                )
        while a_queue:
            a_chunk(i + 1, a_queue.pop(0), "act")
        del v_t[i], vt_t[i]

    load_v(0, ranges=((0, 256), (256, 768), (768, 1536), (1536, N)))
    for i in range(nsamp):
        emit(i)


_nc_cache = {}


def _build(reps=1):
    if reps in _nc_cache:
        return _nc_cache[reps]
    nc = bacc.Bacc("TRN2", target_bir_lowering=False, debug=False)
    x_d = nc.dram_tensor("x", [SPC, C, N], FP32, kind="ExternalInput")
    g_d = nc.dram_tensor("gamma", [1], FP32, kind="ExternalInput")
    o_d = nc.dram_tensor("out", [SPC, C, N], FP32, kind="ExternalOutput")
    with tile.TileContext(nc) as tc, ExitStack() as ctx:
        _emit(tc, ctx, x_d.ap(), g_d.ap(), o_d.ap(), reps=reps)
    nc.compile()
    _nc_cache[reps] = nc
    return nc


def _bench_fn(reps, x, gamma):
    """Build a jitted 8-core executor for the reps-times-repeated kernel with
    device-resident inputs.  Used by test.py for differential timing."""
    import jax
    from jax.experimental.shard_map import shard_map
    from jax.sharding import Mesh, NamedSharding, PartitionSpec

    from concourse import bass2jax

    bass2jax.install_neuronx_cc_hook()
    nc = _build(reps=reps)
    pid = nc.partition_id_tensor.name if nc.partition_id_tensor else None
    in_names, out_names, out_avals, zero_outs = [], [], [], []
    for alloc in nc.m.functions[0].allocations:
        if not isinstance(alloc, mybir.MemoryLocationSet):
            continue
        name = alloc.memorylocations[0].name
        if alloc.kind == "ExternalInput":
            if name != pid:
                in_names.append(name)
        elif alloc.kind == "ExternalOutput":
            out_names.append(name)
            shape = tuple(alloc.tensor_shape)
            dtype = mybir.dt.np(alloc.dtype)
            out_avals.append(jax.core.ShapedArray(shape, dtype))
            zero_outs.append(np.zeros(shape, dtype))
    all_in_names = list(in_names) + list(out_names)
    if pid:
        all_in_names.append(pid)

    def _body(*args):
        operands = list(args)
        if pid:
            operands.append(bass2jax.partition_id_tensor())
        return tuple(
            bass2jax._bass_exec_p.bind(
                *operands,
                out_avals=tuple(out_avals),
                in_names=tuple(all_in_names),
                out_names=tuple(out_names),
                lowering_input_output_aliases=(),
                sim_require_finite=True,
                sim_require_nnan=True,
                nc=nc,
            )
        )

    devices = jax.devices()[:NCORES]
    mesh = Mesh(np.asarray(devices), ("core",))
    specs = (PartitionSpec("core"),) * (len(in_names) + len(out_names))
    fn = jax.jit(
        shard_map(
            _body,
            mesh=mesh,
            in_specs=specs,
            out_specs=(PartitionSpec("core"),) * len(out_names),
            check_rep=False,
        ),
        keep_unused=True,
    )
    sh = NamedSharding(mesh, PartitionSpec("core"))
    ins = {
        "x": np.ascontiguousarray(x, dtype=np.float32).reshape(B, C, N),
        "gamma": np.tile(np.ascontiguousarray(gamma, dtype=np.float32), (NCORES,)),
    }
    args = [jax.device_put(ins[n], sh) for n in in_names]
    args += [
        jax.device_put(np.zeros((NCORES * z.shape[0], *z.shape[1:]), z.dtype), sh)
        for z in zero_outs
    ]
    return fn, args


def kernel(x: np.ndarray, gamma: np.ndarray, **run_kwargs) -> np.ndarray:
    assert x.shape == (B, C, H, W), x.shape
    nc = _build()
    xr = np.ascontiguousarray(x, dtype=np.float32).reshape(B, C, N)
    g = np.ascontiguousarray(gamma, dtype=np.float32)
    in_maps = [
        {"x": xr[g_idx * SPC : (g_idx + 1) * SPC], "gamma": g}
        for g_idx in range(NCORES)
    ]
    res = run_bass_kernel_spmd(nc, in_maps, core_ids=list(range(NCORES)), **run_kwargs)
    outs = [res.results[g_idx]["out"] for g_idx in range(NCORES)]
    full = np.concatenate(outs, axis=0).reshape(B, C, H, W).astype(np.float32)
    if run_kwargs:
        kernel.last_results = res
    return full
